# revision 32
# baseline (speedup 1.0000x reference)
"""Trainium2 Bass kernel for nn_Denoiser_73598559584966.

Full-sequence self-attention (Q=K=V, no scaling) over x: [4, 16, 16, 16, 64]
  t = x.reshape(B, 4096, 64); out = softmax(t @ t^T) @ t
Sharding: 8 cores = 4 batches x 2 query-halves. Each core: 2048 queries
vs the full 4096 keys/values of its batch. No collectives.

Device algorithm (single-pass bf16 scores; scores kept transposed
[keys, queries]; two decoupled 512-query half-pipelines per key tile):
  S'' = (K|1)^T (Q | 88.0-bias)    one bf16 matmul per (key-tile, half),
        contraction 65 = 64 channels + per-query bias row. bias_i ~ |q_i|^2
        so p_max ~ 1; the +88.0 pre-biases for the schraudolph path.
  P = exp(S'' - 88.0)              half 0 -> ScalarE ACT (exact exp,
        per-partition bias AP = -88.0); half 1 -> VectorE single
        tensor_scalar: i16 = max(S''*(128/ln2), 0), whose int16 bits
        reinterpreted as bf16 ARE exp(S''-88) to ~3% (Schraudolph);
        the max-0 clamp maps underflow to +0.0 exactly.
  O^T[65, q] += (V_kt|1)^T P_kt    bf16, accumulated in PSUM; row 64 = sum(P)
Host epilogue divides rows 0..63 by row 64 and transposes while gathering.
"""
import math
import numpy as np

B_, D_, H_, W_, C_ = 4, 16, 16, 16, 64
NTOK = D_ * H_ * W_          # 4096 tokens per batch
NQ = NTOK // 2               # 2048 queries per core
NCORES = 8
NKT = NTOK // 128            # 32 key tiles
CHW = 512                    # queries per chunk (PSUM bank width)
NCP = 2                      # chunk-pairs (1024 queries each)
NG = 4                       # DMA groups over key tiles
GKT = NKT // NG              # 8 key tiles per group

# Schraudolph constants in bf16-bit space.
A_EXP = 128.0 / math.log(2.0)                       # 184.665
SHIFT = (16256.0 - 366393.0 / 65536.0) / A_EXP      # 87.99942

_CACHE = {}


def _build_nc():
    import concourse.bacc as bacc
    import concourse.mybir as mybir
    from concourse.tile import TileContext

    f32 = mybir.dt.float32
    bf16 = mybir.dt.bfloat16
    i16 = mybir.dt.int16
    EXP = mybir.ActivationFunctionType.Exp
    MULT = mybir.AluOpType.mult
    MAX = mybir.AluOpType.max
    nc = bacc.Bacc("TRN2", target_bir_lowering=False, debug=False)

    kq = nc.dram_tensor("kq", [65, NTOK], bf16, kind="ExternalInput")
    qb = nc.dram_tensor("qb", [65, NQ], bf16, kind="ExternalInput")
    vpk = nc.dram_tensor("vpk", [128, NKT * 65], bf16, kind="ExternalInput")
    out = nc.dram_tensor("out", [65, NQ], bf16, kind="ExternalOutput")

    GW = GKT * 128            # tokens per kq DMA group
    with TileContext(nc) as tc:
        with (
            tc.tile_pool(name="pp", bufs=4) as pp,
            tc.tile_pool(name="ps_s", bufs=3, space="PSUM") as ps_s,
        ):
            const = sbo = pp   # merged pools: fewer context-exit barriers
            ps_o = ps_s
            # ---- PE + ACT warmup during the DMA prefix ----
            # N=512 matmuls, >=3.4us of contiguous PE activity: shorter or
            # narrower warmup bursts leave the HAM clock-gate cold (measured:
            # the whole main loop then runs at ~1/2 clock). Operands come
            # from the framework's pre-barrier bf16 const tensor, so the PE
            # starts the moment the init barrier releases (no memset wait).
            one_w = nc.const_aps.tensor(1.0, (128, 128), bf16)
            one_m = nc.const_aps.tensor(1.0, (128, CHW), bf16)
            one_s = nc.const_aps.tensor(1.0, (128, 1), bf16)
            bshift = const.tile([128, 1], f32, tag="bshift", bufs=1)
            nc.vector.memset(bshift, -SHIFT)
            wexp = const.tile([128, 1], f32, tag="wexp", bufs=1)
            nc.scalar.activation(wexp, one_s, EXP)  # pull exp table load
            for i in range(8):
                wps = ps_s.tile([128, CHW], f32, tag="s0" if i % 2 == 0 else "s1")
                nc.tensor.matmul(wps, one_w, one_m, start=True, stop=True)

            # ---- input DMAs ----
            qb_t = const.tile([65, NQ], bf16, tag="qb", bufs=1)
            kq_t = const.tile([65, NTOK], bf16, tag="kq", bufs=1)
            vpk_t = const.tile([128, NKT * 65], bf16, tag="vpk", bufs=1)
            # tranche 0 (~200KB): just enough for the first ~8 key tiles
            nc.sync.dma_start(out=kq_t[:, 0:GW], in_=kq[:, 0:GW])
            nc.sync.dma_start(out=qb_t[:, 0:512], in_=qb[:, 0:512])
            nc.sync.dma_start(out=qb_t[:, 512:1024], in_=qb[:, 512:1024])
            nc.sync.dma_start(out=vpk_t[:, 0:GKT * 65],
                              in_=vpk[:, 0:GKT * 65])
            for g in range(1, NG):
                nc.sync.dma_start(out=kq_t[:, g * GW:(g + 1) * GW],
                                  in_=kq[:, g * GW:(g + 1) * GW])
                nc.sync.dma_start(
                    out=vpk_t[:, g * GKT * 65:(g + 1) * GKT * 65],
                    in_=vpk[:, g * GKT * 65:(g + 1) * GKT * 65])
                if g == 1:
                    nc.sync.dma_start(out=qb_t[:, 1024:2048],
                                      in_=qb[:, 1024:2048])

            # ---- main loop: 2 chunk-pairs x 32 key tiles ----
            for cp in range(NCP):
                q0 = slice(cp * 1024, cp * 1024 + 512)
                q1 = slice(cp * 1024 + 512, cp * 1024 + 1024)
                o0 = ps_o.tile([65, CHW], f32, tag="o0", bufs=1)
                o1 = ps_o.tile([65, CHW], f32, tag="o1", bufs=1)
                # Software-pipelined: PV for key tile kt-1 is emitted after
                # the S matmuls of kt, so the PV pair runs back-to-back with
                # its exp results long since ready.
                prev = None
                for kt in range(NKT):
                    ks = slice(kt * 128, (kt + 1) * 128)
                    s0 = ps_s.tile([128, CHW], f32, tag="s0")
                    s1 = ps_s.tile([128, CHW], f32, tag="s1")
                    nc.tensor.matmul(s0, kq_t[:, ks], qb_t[:, q0],
                                     start=True, stop=True)
                    nc.tensor.matmul(s1, kq_t[:, ks], qb_t[:, q1],
                                     start=True, stop=True)
                    if prev is not None:
                        pp0, pp1, pkt = prev
                        vs = slice(pkt * 65, pkt * 65 + 65)
                        nc.tensor.matmul(
                            o0, vpk_t[:, vs], pp0, start=(pkt == 0),
                            stop=False, skip_group_check=True)
                        nc.tensor.matmul(
                            o1, vpk_t[:, vs], pp1, start=(pkt == 0),
                            stop=False, skip_group_check=True)
                    p0 = pp.tile([128, CHW], bf16, tag="p0")
                    p1 = pp.tile([128, CHW], bf16, tag="p1")
                    nc.scalar.activation(p0, s0, EXP, bias=bshift)
                    nc.vector.tensor_scalar(
                        p1.bitcast(i16), s1, A_EXP, 0.0, MULT, MAX)
                    prev = (p0, p1, kt)
                pp0, pp1, pkt = prev
                vs = slice(pkt * 65, pkt * 65 + 65)
                nc.tensor.matmul(o0, vpk_t[:, vs], pp0, start=False,
                                 stop=True, skip_group_check=True)
                nc.tensor.matmul(o1, vpk_t[:, vs], pp1, start=False,
                                 stop=True, skip_group_check=True)
                # ---- ship O^T (normalize + transpose on host) ----
                osb = sbo.tile([65, 2 * CHW], bf16, tag="osb", bufs=2)
                nc.scalar.copy(osb[:, 0:CHW], o0)
                nc.sync.dma_start(out=out[:, q0], in_=osb[:, 0:CHW])
                nc.vector.tensor_copy(osb[:, CHW:2 * CHW], o1)
                nc.sync.dma_start(out=out[:, q1], in_=osb[:, CHW:2 * CHW])
    nc.compile()
    return nc


def _prep_inputs(x):
    """Host-side shard + operand marshaling. Returns list of 8 in_maps."""
    import ml_dtypes
    bf16 = ml_dtypes.bfloat16
    t = np.ascontiguousarray(x, np.float32).reshape(B_, NTOK, C_)
    in_maps = []
    for b in range(B_):
        kv = t[b]                                   # [4096, 64]
        kmax = float(np.linalg.norm(kv.astype(np.float64), axis=1).max())
        kq = np.concatenate(
            [kv.T, np.ones((1, NTOK), np.float32)]).astype(bf16)
        vpk = np.concatenate(
            [np.concatenate([kv[i * 128:(i + 1) * 128],
                             np.ones((128, 1), np.float32)], axis=1)
             for i in range(NKT)], axis=1).astype(bf16)   # [128, 32*65]
        for h in range(2):
            q = t[b, h * NQ:(h + 1) * NQ]           # [2048, 64]
            qn = np.linalg.norm(q.astype(np.float64), axis=1)
            bias = np.minimum(
                np.maximum(qn * qn + 0.5, qn * kmax - 80.0), 130.0)
            brow = (np.float32(SHIFT) - bias.astype(np.float32))
            qb = np.concatenate([q.T, brow[None, :]]).astype(bf16)
            in_maps.append({"kq": kq, "qb": qb, "vpk": vpk})
    return in_maps


def run(x, trace=False):
    from concourse.bass_utils import run_bass_kernel_spmd
    if "nc" not in _CACHE:
        _CACHE["nc"] = _build_nc()
    nc = _CACHE["nc"]
    in_maps = _prep_inputs(x)
    res = run_bass_kernel_spmd(
        nc, in_maps, core_ids=list(range(NCORES)), trace=trace,
    )
    full = np.empty((B_, NTOK, C_), np.float32)
    for b in range(B_):
        for h in range(2):
            o = res.results[2 * b + h]["out"].astype(np.float32)  # [65, 2048]
            full[b, h * NQ:(h + 1) * NQ] = (o[0:C_] / o[C_]).T
    return full.reshape(B_, D_, H_, W_, C_), res


def kernel(x):
    out, _ = run(x, trace=False)
    return out
